# revision 24
# baseline (speedup 1.0000x reference)
"""CFormerAdapter (CIF + cif_proj + RMSNorm + text_proj) on 8 TRN2 NeuronCores.

Sharding: pure data parallel — batch 16 -> 2 per core; weights replicated.

Device algorithm (per core, 2 batch elements):
  The CIF recurrence is replaced by an exact closed form: with C = cumsum(alpha),
  the scatter weights are W[n,t] = relu(min(C_t, n+1) - max(C_{t-1}, n)) with C
  clamped at num_tokens (token nt-1 absorbs the tail; tokens >= nt get 0).
  cumsum is computed as a lower-triangular fp32 matmul per 128-frame chunk plus
  an exclusive scan of chunk totals.  The einsum / cif_proj / text_proj run as
  fp32r (full-rate fp32) matmuls; RMSNorm uses DVE reciprocal + ACT sqrt.
"""

import os
import numpy as np
from contextlib import ExitStack

B, T, D = 16, 1500, 1024
TP = 1536                  # host-padded frame count (12 x 128)
DTXT = 4096
N = 375
NPAD = 384
NCORES = 8
BPC = B // NCORES          # batches per core
KC = 12                    # 128-frame chunks
DS = 8                     # d slices of 128 over 1023 (last is 127)
ET = 8                     # e tiles of 128 over 1024
RMS_EPS = 1e-6

_NC = None
_last_exec_ns = None


def _build_module():
    import concourse.bass as bass
    from concourse import bacc, mybir
    import concourse.tile as tile

    f32 = mybir.dt.float32
    f32r = mybir.dt.float32r
    i32 = mybir.dt.int32
    AF = mybir.ActivationFunctionType
    OP = mybir.AluOpType

    nc = bacc.Bacc("TRN2", target_bir_lowering=False, debug=False)
    audio = nc.dram_tensor("audio", [BPC, TP, D], f32, kind="ExternalInput").ap()
    alphaT = nc.dram_tensor("alphaT", [BPC, 128, KC], f32, kind="ExternalInput").ap()
    cifwT = nc.dram_tensor("cifwT", [1023, 1024], f32, kind="ExternalInput").ap()
    cifbpe = nc.dram_tensor("cifbpe", [128, ET], f32, kind="ExternalInput").ap()
    lnwpe = nc.dram_tensor("lnwpe", [128, ET], f32, kind="ExternalInput").ap()
    textwT = nc.dram_tensor("textwT", [1024, DTXT], f32, kind="ExternalInput").ap()
    textb = nc.dram_tensor("textb", [1, DTXT], f32, kind="ExternalInput").ap()
    nfb = nc.dram_tensor("nfb", [128, BPC], f32, kind="ExternalInput").ap()
    ntb = nc.dram_tensor("ntb", [128, BPC], f32, kind="ExternalInput").ap()
    out = nc.dram_tensor("out", [BPC, N, DTXT], f32, kind="ExternalOutput").ap()
    npred = nc.dram_tensor("npred", [BPC, 1], f32, kind="ExternalOutput").ap()

    with tile.TileContext(nc) as tc, ExitStack() as ctx:
        const = ctx.enter_context(tc.tile_pool(name="const", bufs=1))
        cwp = ctx.enter_context(tc.tile_pool(name="cifw", bufs=8))
        twp = ctx.enter_context(tc.tile_pool(name="textw", bufs=12))
        audp = ctx.enter_context(tc.tile_pool(name="aud", bufs=3))
        wtp = ctx.enter_context(tc.tile_pool(name="wt", bufs=4))
        htp = ctx.enter_context(tc.tile_pool(name="ht", bufs=16))
        y1p = ctx.enter_context(tc.tile_pool(name="y1", bufs=8))
        y2p = ctx.enter_context(tc.tile_pool(name="y2", bufs=16))
        sqp = ctx.enter_context(tc.tile_pool(name="sq", bufs=6))
        smp = ctx.enter_context(tc.tile_pool(name="sm", bufs=1))
        stg = ctx.enter_context(tc.tile_pool(name="stage", bufs=4))
        psp = ctx.enter_context(tc.tile_pool(name="ps", bufs=8, space="PSUM"))

        # ---------------- constants ----------------
        iok = const.tile([128, 128], i32)
        nc.gpsimd.iota(iok[:], pattern=[[0, 128]], base=0, channel_multiplier=1)
        iom = const.tile([128, 128], i32)
        nc.gpsimd.iota(iom[:], pattern=[[1, 128]], base=0, channel_multiplier=0)
        tri = const.tile([128, 128], f32)
        nc.vector.tensor_tensor(tri[:], iok[:], iom[:], OP.is_le)

        iot_i = const.tile([128, KC], i32)
        nc.gpsimd.iota(iot_i[:], pattern=[[128, KC]], base=0, channel_multiplier=1)
        iota_t = const.tile([128, KC], f32)
        nc.vector.tensor_copy(iota_t[:], iot_i[:])

        ion_i = const.tile([128, NPAD], i32)
        nc.gpsimd.iota(ion_i[:], pattern=[[1, NPAD]], base=0, channel_multiplier=0)
        iota_n = const.tile([128, NPAD], f32)
        nc.vector.tensor_copy(iota_n[:], ion_i[:])
        iota_n1 = const.tile([128, NPAD], f32)
        nc.vector.tensor_scalar(iota_n1[:], iota_n[:], 1.0, None, OP.add)

        ones_row = const.tile([1, 128], f32)
        nc.vector.memset(ones_row[:], 1.0)
        ones_row_r = const.tile([1, 128], f32r)
        nc.vector.tensor_copy(ones_row_r[:], ones_row[:])
        ones_col = const.tile([128, 1], f32)
        nc.vector.memset(ones_col[:], 1.0)
        ones_col_r = const.tile([128, 1], f32r)
        nc.vector.tensor_copy(ones_col_r[:], ones_col[:])
        zero_row = const.tile([1, KC], f32)
        nc.vector.memset(zero_row[:], 0.0)

        cifb_cols = const.tile([128, ET], f32)
        nc.sync.dma_start(cifb_cols[:], cifbpe[:])
        lnw_cols = const.tile([128, ET], f32)
        nc.sync.dma_start(lnw_cols[:], lnwpe[:])
        nfb_t = const.tile([128, BPC], f32)
        nc.sync.dma_start(nfb_t[:], nfb[:])
        ntb_t = const.tile([128, BPC], f32)
        nc.sync.dma_start(ntb_t[:], ntb[:])

        # critical-path prep DMAs first (before bulk weight loads)
        at_l = []
        for b in range(BPC):
            at = smp.tile([128, KC], f32, tag=f"at{b}", name=f"at{b}")
            nc.scalar.dma_start(at[:], alphaT[b])
            at_l.append(at)

        y2_t = [[None] * ET for _ in range(BPC)]
        ct_l, cpt_l = [], []
        cifw_t = []

        # ================= phase: alpha prep (both batches) =================
        for b in range(BPC):
            at = at_l[b]
            asig = smp.tile([128, KC], f32, tag=f"asig{b}")
            nc.scalar.activation(asig[:], at[:], AF.Sigmoid)
            am = smp.tile([128, KC], f32, tag=f"am{b}")
            nc.vector.scalar_tensor_tensor(
                am[:], iota_t[:], nfb_t[:, b:b + 1], asig[:], OP.is_lt, OP.mult)

            # in-chunk inclusive prefix via triangular fp32 matmul
            psC = psp.tile([128, KC], f32, tag="acc", name=f"psC{b}")
            nc.tensor.matmul(psC[:], tri[:], am[:], start=True, stop=True)

            # chunk totals (partition 0) -> inclusive scan -> exclusive offsets
            psT = psp.tile([1, KC], f32, tag="acc", name=f"psT{b}")
            nc.tensor.matmul(psT[:], ones_col[:], am[:], start=True, stop=True)
            scani = smp.tile([1, KC], f32, tag=f"scani{b}")
            nc.vector.tensor_tensor_scan(
                scani[:], psT[:], zero_row[:], 0.0, OP.add, OP.add)
            nc.sync.dma_start(npred[b:b + 1, :], scani[0:1, KC - 1:KC])
            offs = smp.tile([1, KC], f32, tag=f"offs{b}")
            nc.vector.memset(offs[0:1, 0:1], 0.0)
            nc.vector.tensor_copy(offs[0:1, 1:KC], scani[0:1, 0:KC - 1])

            # scale = ntk / total  (reciprocal + mult; ntk read from ntb row 0)
            rcp = smp.tile([1, 1], f32, tag=f"rcp{b}")
            nc.vector.reciprocal(rcp[:], scani[0:1, KC - 1:KC])
            scal = smp.tile([1, 1], f32, tag=f"scal{b}")
            nc.vector.tensor_tensor(scal[:], ntb_t[0:1, b:b + 1], rcp[:], OP.mult)

            # broadcast offs and scale to 128 partitions (exact fp32 K=1 matmuls)
            psOf = psp.tile([128, KC], f32, tag="acc", name=f"psOf{b}")
            nc.tensor.matmul(psOf[:], ones_row[:], offs[:], start=True, stop=True)
            offs_b = smp.tile([128, KC], f32, tag=f"offsb{b}")
            nc.vector.tensor_copy(offs_b[:], psOf[:])
            psSc = psp.tile([128, 1], f32, tag="acc", name=f"psSc{b}")
            nc.tensor.matmul(psSc[:], ones_row[:], scal[:], start=True, stop=True)
            sc_b = smp.tile([128, 1], f32, tag=f"scb{b}")
            nc.vector.tensor_copy(sc_b[:], psSc[:])

            # full cumsum, scaled and clamped at ntk
            cf = smp.tile([128, KC], f32, tag=f"cf{b}")
            nc.vector.tensor_tensor(cf[:], psC[:], offs_b[:], OP.add)
            ct = smp.tile([128, KC], f32, tag=f"ct{b}")
            nc.vector.tensor_scalar(ct[:], cf[:], sc_b[:], ntb_t[:, b:b + 1],
                                    OP.mult, OP.min)
            ams = smp.tile([128, KC], f32, tag=f"ams{b}")
            nc.vector.tensor_scalar_mul(ams[:], am[:], sc_b[:])
            cm = smp.tile([128, KC], f32, tag=f"cm{b}")
            nc.vector.scalar_tensor_tensor(
                cm[:], cf[:], sc_b[:], ams[:], OP.mult, OP.subtract)
            cpt = smp.tile([128, KC], f32, tag=f"cpt{b}")
            nc.vector.tensor_scalar_min(cpt[:], cm[:], ntb_t[:, b:b + 1])
            ct_l.append(ct)
            cpt_l.append(cpt)

        # ===== einsum / cif / rms interleaved across batches for PE overlap ====
        ht_l = [None] * BPC
        psY_l = [None] * BPC
        y1_l = [None] * BPC
        psS_l = [None] * BPC

        def einsum_phase(b):
            psH = []
            for d in range(DS):
                psH.append(psp.tile([128, NPAD], f32, tag="acc",
                                    name=f"psH{b}_{d}"))
            for c in range(KC):
                audc = audp.tile([128, 1024], f32r, tag="audc",
                                 name=f"audc{b}_{c}")
                for h in range(2):
                    nc.sync.dma_start(
                        audc[64 * h:64 * (h + 1), :],
                        audio[b, 128 * c + 64 * h:128 * c + 64 * (h + 1),
                              :].bitcast(f32r))
                wa = wtp.tile([128, NPAD], f32, tag="wa", name=f"wa{b}_{c}")
                nc.vector.tensor_scalar_min(wa[:], iota_n1[:], ct_l[b][:, c:c + 1])
                wd = wtp.tile([128, NPAD], f32, tag="wd", name=f"wd{b}_{c}")
                nc.vector.scalar_tensor_tensor(
                    wd[:], iota_n[:], cpt_l[b][:, c:c + 1], wa[:],
                    OP.max, OP.subtract)
                wr = wtp.tile([128, NPAD], f32r, tag="wr", bufs=6, name=f"wr{b}_{c}")
                nc.scalar.activation(wr[:], wd[:], AF.Relu, scale=-1.0)
                for d in range(DS):
                    dw = 128 if d < DS - 1 else 127
                    nc.tensor.matmul(
                        psH[d][0:dw, :], audc[:, 128 * d:128 * d + dw], wr[:],
                        start=(c == 0), stop=(c == KC - 1))
            ht = []
            for d in range(DS):
                dw = 128 if d < DS - 1 else 127
                t = htp.tile([128, NPAD], f32r, tag="ht", name=f"ht{b}_{d}")
                nc.scalar.activation(t[0:dw, :], psH[d][0:dw, :], AF.Identity)
                ht.append(t)
            ht_l[b] = ht

        def cif_phase(b):
            psY = []
            for e in range(ET):
                psY.append(psp.tile([128, NPAD], f32, tag="acc",
                                    name=f"psY{b}_{e}"))
            # d-outer: the first matmuls need only cifw_t[0], so cif streams
            # against the cif-weight DMAs instead of waiting for all of them
            for d in range(DS):
                dw = 128 if d < DS - 1 else 127
                for e in range(ET):
                    nc.tensor.matmul(
                        psY[e][:], cifw_t[d][0:dw, 128 * e:128 * (e + 1)],
                        ht_l[b][d][0:dw, :],
                        start=(d == 0), stop=(d == DS - 1))
            psY_l[b] = psY

        def ss_phase(b):
            y1 = []
            psS = None
            for e in range(ET):
                t = y1p.tile([128, NPAD], f32, tag="y1", name=f"y1_{b}_{e}")
                nc.scalar.activation(t[:], psY_l[b][e][:], AF.Identity,
                                     bias=cifb_cols[:, e:e + 1])
                y1.append(t)
                sq = sqp.tile([128, NPAD], f32r, tag="sq", name=f"sq{b}_{e}")
                nc.scalar.square(sq[:], t[:])
                if e == 0:
                    psS = psp.tile([1, NPAD], f32, tag="acc", name=f"psS{b}")
                nc.tensor.matmul(psS[:], ones_col_r[:], sq[:],
                                 start=(e == 0), stop=(e == ET - 1))
            y1_l[b] = y1
            psS_l[b] = psS
            msq = smp.tile([1, NPAD], f32, tag="msq", name=f"msq{b}")
            nc.vector.tensor_scalar(msq[:], psS[:], 1.0 / 1024.0, RMS_EPS,
                                    OP.mult, OP.add)
            sd = smp.tile([1, NPAD], f32, tag="sd", name=f"sd{b}")
            nc.scalar.activation(sd[:], msq[:], AF.Sqrt)
            rstd_f = smp.tile([1, NPAD], f32, tag="rstdf", name=f"rstdf{b}")
            nc.vector.reciprocal(rstd_f[:], sd[:])
            rstd = smp.tile([1, NPAD], f32r, tag="rstd", name=f"rstd{b}")
            nc.vector.tensor_copy(rstd[:], rstd_f[:])
            return rstd

        def y2_phase(b, rstd):
            psR = psp.tile([128, NPAD], f32, tag="acc", name=f"psR{b}")
            nc.tensor.matmul(psR[:], ones_row_r[:], rstd[:], start=True, stop=True)
            for e in range(ET):
                t = y2p.tile([128, NPAD], f32r, tag="y2", name=f"y2_{b}_{e}")
                nc.vector.scalar_tensor_tensor(
                    t[:], y1_l[b][e][:], lnw_cols[:, e:e + 1], psR[:],
                    OP.mult, OP.mult)
                y2_t[b][e] = t

        einsum_phase(0)
        # cif weights resident (f32r via bitcast byte copy); emitted after the
        # first einsum so its audio-chunk DMAs get the head of the queues
        for d in range(DS):
            dw = 128 if d < DS - 1 else 127
            t = cwp.tile([128, 1024], f32r, tag="cw", name=f"cw{d}")
            nc.sync.dma_start(t[0:dw, :],
                              cifwT[128 * d:128 * d + dw, :].bitcast(f32r))
            cifw_t.append(t)
        cif_phase(0)
        rstd0 = ss_phase(0)
        einsum_phase(1)
        y2_phase(0, rstd0)
        cif_phase(1)
        rstd1 = ss_phase(1)

        # ================= phase: text_proj, 512-wide f chunks =================
        NT_TILES = (128, 128, N - 256)
        tw_cache = {}
        bias_cache = {}

        def text_fcg(fcg, bs):
            f0 = 512 * fcg
            if fcg not in tw_cache:
                tw = []
                for e in range(ET):
                    t = twp.tile([128, 512], f32r, tag="tw",
                                 name=f"tw{fcg}_{e}")
                    nc.sync.dma_start(t[:], textwT[128 * e:128 * (e + 1),
                                                   f0:f0 + 512].bitcast(f32r))
                    tw.append(t)
                tw_cache[fcg] = tw
                tbq = twp.tile([1, 512], f32r, tag="tbq", bufs=2,
                               name=f"tbq{fcg}")
                nc.scalar.dma_start(tbq[:], textb[0:1, f0:f0 + 512].bitcast(f32r))
                psB = psp.tile([128, 512], f32, tag="acc", name=f"psB{fcg}")
                nc.tensor.matmul(psB[:], ones_row_r[:], tbq[:],
                                 start=True, stop=True)
                bias_sb = stg.tile([128, 512], f32, tag="bias", bufs=3,
                                   name=f"bias{fcg}")
                nc.vector.tensor_copy(bias_sb[:], psB[:])
                bias_cache[fcg] = bias_sb
            tw = tw_cache[fcg]
            bias_sb = bias_cache[fcg]
            for b in bs:
                for ntile in range(3):
                    nw = NT_TILES[ntile]
                    psO = psp.tile([128, 512], f32, tag="acc",
                                   name=f"psO{fcg}_{b}_{ntile}")
                    for e in range(ET):
                        nc.tensor.matmul(
                            psO[0:nw, :],
                            y2_t[b][e][:, 128 * ntile:128 * ntile + nw],
                            tw[e][:],
                            start=(e == 0), stop=(e == ET - 1))
                    so = stg.tile([128, 512], f32, tag="so",
                                  name=f"so{fcg}_{b}_{ntile}")
                    nc.vector.tensor_tensor(
                        so[0:nw, :], psO[0:nw, :], bias_sb[0:nw, :], OP.add)
                    nc.sync.dma_start(
                        out[b, 128 * ntile:128 * ntile + nw, f0:f0 + 512],
                        so[0:nw, :])

        # first text group for batch 0 runs between ss(1) and y2(1): its
        # matmuls hide the second RMS reciprocal chain on the in-order PE
        text_fcg(0, [0])
        y2_phase(1, rstd1)
        text_fcg(0, [1])
        for fcg in range(1, 8):
            text_fcg(fcg, [0, 1])
    nc.compile()
    return nc


def _get_module():
    global _NC
    if _NC is None:
        _NC = _build_module()
    return _NC


def kernel(**inputs):
    global _last_exec_ns
    from concourse import bass_utils

    audio = np.ascontiguousarray(np.asarray(inputs["audio_features"], dtype=np.float32))
    cif_w = np.asarray(inputs["cif_w"], dtype=np.float32)
    cif_b = np.asarray(inputs["cif_b"], dtype=np.float32)
    ln_w = np.asarray(inputs["ln_w"], dtype=np.float32)
    text_w = np.asarray(inputs["text_w"], dtype=np.float32)
    text_b = np.ascontiguousarray(
        np.asarray(inputs["text_b"], dtype=np.float32).reshape(1, DTXT))
    num_frames = np.asarray(inputs["num_frames"])
    num_text_tokens = np.asarray(inputs["num_text_tokens"])

    cifwT = np.ascontiguousarray(cif_w.T)
    textwT = np.ascontiguousarray(text_w.T)
    cifbpe = np.ascontiguousarray(cif_b.reshape(ET, 128).T)
    lnwpe = np.ascontiguousarray(ln_w.reshape(ET, 128).T)
    nf32 = num_frames.astype(np.float32)
    nt32 = num_text_tokens.astype(np.float32)

    audio_p = np.concatenate(
        [audio, np.zeros((B, TP - T, D), dtype=np.float32)], axis=1)
    # alpha channel, per batch laid out as [128, KC] (partition = t % 128)
    alphaT = np.ascontiguousarray(
        audio_p[:, :, D - 1].reshape(B, KC, 128).transpose(0, 2, 1))

    in_maps = []
    for c in range(NCORES):
        b0 = BPC * c
        nfb = np.ascontiguousarray(
            np.broadcast_to(nf32[b0:b0 + BPC][None, :], (128, BPC)))
        ntb = np.ascontiguousarray(
            np.broadcast_to(nt32[b0:b0 + BPC][None, :], (128, BPC)))
        in_maps.append({
            "audio": np.ascontiguousarray(audio_p[b0:b0 + BPC]),
            "alphaT": np.ascontiguousarray(alphaT[b0:b0 + BPC]),
            "cifwT": cifwT,
            "cifbpe": cifbpe,
            "lnwpe": lnwpe,
            "textwT": textwT,
            "textb": text_b,
            "nfb": nfb,
            "ntb": ntb,
        })

    nc = _get_module()
    trace = bool(int(os.environ.get("KERNEL_TRACE", "0")))
    if trace:
        import sys
        import types
        import trn_agent_boot.trn_boot as tb
        hookmod = types.ModuleType("antenv.axon_hooks")
        hook = tb._ntff_profile_via_ctypes('/opt/axon/libaxon_pjrt.so')
        hookmod.get_axon_ntff_profile_hook = lambda: hook
        sys.modules["antenv.axon_hooks"] = hookmod
    res = bass_utils.run_bass_kernel_spmd(
        nc, in_maps, core_ids=list(range(NCORES)), trace=trace)
    _last_exec_ns = res.exec_time_ns

    out = np.concatenate([r["out"] for r in res.results], axis=0)
    npred = np.concatenate(
        [r["npred"].reshape(BPC) for r in res.results], axis=0)
    return out, num_text_tokens.copy(), npred


# revision 25
# speedup vs baseline: 1.0789x; 1.0789x over previous
"""CFormerAdapter (CIF + cif_proj + RMSNorm + text_proj) on 8 TRN2 NeuronCores.

Sharding: pure data parallel — batch 16 -> 2 per core; weights replicated.

Device algorithm (per core, 2 batch elements):
  The CIF recurrence is replaced by an exact closed form: with C = cumsum(alpha),
  the scatter weights are W[n,t] = relu(min(C_t, n+1) - max(C_{t-1}, n)) with C
  clamped at num_tokens (token nt-1 absorbs the tail; tokens >= nt get 0).
  cumsum is computed as a lower-triangular fp32 matmul per 128-frame chunk plus
  an exclusive scan of chunk totals.  The einsum / cif_proj / text_proj run as
  fp32r (full-rate fp32) matmuls; RMSNorm uses DVE reciprocal + ACT sqrt.
"""

import os
import numpy as np
from contextlib import ExitStack

B, T, D = 16, 1500, 1024
TP = 1536                  # host-padded frame count (12 x 128)
DTXT = 4096
N = 375
NPAD = 384
NCORES = 8
BPC = B // NCORES          # batches per core
KC = 12                    # 128-frame chunks
DS = 8                     # d slices of 128 over 1023 (last is 127)
ET = 8                     # e tiles of 128 over 1024
RMS_EPS = 1e-6

_NC = None
_last_exec_ns = None


def _build_module():
    import concourse.bass as bass
    from concourse import bacc, mybir
    import concourse.tile as tile

    f32 = mybir.dt.float32
    f32r = mybir.dt.float32r
    i32 = mybir.dt.int32
    AF = mybir.ActivationFunctionType
    OP = mybir.AluOpType

    nc = bacc.Bacc("TRN2", target_bir_lowering=False, debug=False)
    audio = nc.dram_tensor("audio", [BPC, TP, D], f32, kind="ExternalInput").ap()
    alphaT = nc.dram_tensor("alphaT", [BPC, 128, KC], f32, kind="ExternalInput").ap()
    cifwT = nc.dram_tensor("cifwT", [1023, 1024], f32, kind="ExternalInput").ap()
    cifbpe = nc.dram_tensor("cifbpe", [128, ET], f32, kind="ExternalInput").ap()
    lnwpe = nc.dram_tensor("lnwpe", [128, ET], f32, kind="ExternalInput").ap()
    textwT = nc.dram_tensor("textwT", [1024, DTXT], f32, kind="ExternalInput").ap()
    textb = nc.dram_tensor("textb", [1, DTXT], f32, kind="ExternalInput").ap()
    nfb = nc.dram_tensor("nfb", [128, BPC], f32, kind="ExternalInput").ap()
    ntb = nc.dram_tensor("ntb", [128, BPC], f32, kind="ExternalInput").ap()
    out = nc.dram_tensor("out", [BPC, N, DTXT], f32, kind="ExternalOutput").ap()
    npred = nc.dram_tensor("npred", [BPC, 1], f32, kind="ExternalOutput").ap()

    with tile.TileContext(nc) as tc, ExitStack() as ctx:
        const = ctx.enter_context(tc.tile_pool(name="const", bufs=1))
        cwp = ctx.enter_context(tc.tile_pool(name="cifw", bufs=8))
        twp = ctx.enter_context(tc.tile_pool(name="textw", bufs=12))
        audp = ctx.enter_context(tc.tile_pool(name="aud", bufs=3))
        wtp = ctx.enter_context(tc.tile_pool(name="wt", bufs=4))
        htp = ctx.enter_context(tc.tile_pool(name="ht", bufs=16))
        y1p = ctx.enter_context(tc.tile_pool(name="y1", bufs=8))
        y2p = ctx.enter_context(tc.tile_pool(name="y2", bufs=16))
        sqp = ctx.enter_context(tc.tile_pool(name="sq", bufs=6))
        smp = ctx.enter_context(tc.tile_pool(name="sm", bufs=1))
        stg = ctx.enter_context(tc.tile_pool(name="stage", bufs=4))
        psp = ctx.enter_context(tc.tile_pool(name="ps", bufs=8, space="PSUM"))

        # ---------------- constants ----------------
        iok = const.tile([128, 128], i32)
        nc.gpsimd.iota(iok[:], pattern=[[0, 128]], base=0, channel_multiplier=1)
        iom = const.tile([128, 128], i32)
        nc.gpsimd.iota(iom[:], pattern=[[1, 128]], base=0, channel_multiplier=0)
        tri = const.tile([128, 128], f32)
        nc.vector.tensor_tensor(tri[:], iok[:], iom[:], OP.is_le)

        iot_i = const.tile([128, KC], i32)
        nc.gpsimd.iota(iot_i[:], pattern=[[128, KC]], base=0, channel_multiplier=1)
        iota_t = const.tile([128, KC], f32)
        nc.vector.tensor_copy(iota_t[:], iot_i[:])

        ion_i = const.tile([128, NPAD], i32)
        nc.gpsimd.iota(ion_i[:], pattern=[[1, NPAD]], base=0, channel_multiplier=0)
        iota_n = const.tile([128, NPAD], f32)
        nc.vector.tensor_copy(iota_n[:], ion_i[:])
        iota_n1 = const.tile([128, NPAD], f32)
        nc.vector.tensor_scalar(iota_n1[:], iota_n[:], 1.0, None, OP.add)

        ones_row = const.tile([1, 128], f32)
        nc.vector.memset(ones_row[:], 1.0)
        ones_row_r = const.tile([1, 128], f32r)
        nc.vector.tensor_copy(ones_row_r[:], ones_row[:])
        ones_col = const.tile([128, 1], f32)
        nc.vector.memset(ones_col[:], 1.0)
        ones_col_r = const.tile([128, 1], f32r)
        nc.vector.tensor_copy(ones_col_r[:], ones_col[:])
        zero_row = const.tile([1, KC], f32)
        nc.vector.memset(zero_row[:], 0.0)

        cifb_cols = const.tile([128, ET], f32)
        nc.sync.dma_start(cifb_cols[:], cifbpe[:])
        lnw_cols = const.tile([128, ET], f32)
        nc.sync.dma_start(lnw_cols[:], lnwpe[:])
        nfb_t = const.tile([128, BPC], f32)
        nc.sync.dma_start(nfb_t[:], nfb[:])
        ntb_t = const.tile([128, BPC], f32)
        nc.sync.dma_start(ntb_t[:], ntb[:])

        # critical-path prep DMAs first (before bulk weight loads)
        at_l = []
        for b in range(BPC):
            at = smp.tile([128, KC], f32, tag=f"at{b}", name=f"at{b}")
            nc.scalar.dma_start(at[:], alphaT[b])
            at_l.append(at)

        y2_t = [[None] * ET for _ in range(BPC)]
        ct_l, cpt_l = [], []
        cifw_t = []

        # ================= phase: alpha prep (both batches) =================
        for b in range(BPC):
            at = at_l[b]
            asig = smp.tile([128, KC], f32, tag=f"asig{b}")
            nc.scalar.activation(asig[:], at[:], AF.Sigmoid)
            am = smp.tile([128, KC], f32, tag=f"am{b}")
            nc.vector.scalar_tensor_tensor(
                am[:], iota_t[:], nfb_t[:, b:b + 1], asig[:], OP.is_lt, OP.mult)

            # in-chunk inclusive prefix via triangular fp32 matmul
            psC = psp.tile([128, KC], f32, tag="acc", name=f"psC{b}")
            nc.tensor.matmul(psC[:], tri[:], am[:], start=True, stop=True)

            # chunk totals (partition 0) -> inclusive scan -> exclusive offsets
            psT = psp.tile([1, KC], f32, tag="acc", name=f"psT{b}")
            nc.tensor.matmul(psT[:], ones_col[:], am[:], start=True, stop=True)
            scani = smp.tile([1, KC], f32, tag=f"scani{b}")
            nc.vector.tensor_tensor_scan(
                scani[:], psT[:], zero_row[:], 0.0, OP.add, OP.add)
            nc.sync.dma_start(npred[b:b + 1, :], scani[0:1, KC - 1:KC])
            offs = smp.tile([1, KC], f32, tag=f"offs{b}")
            nc.vector.memset(offs[0:1, 0:1], 0.0)
            nc.vector.tensor_copy(offs[0:1, 1:KC], scani[0:1, 0:KC - 1])

            # scale = ntk / total  (reciprocal + mult; ntk read from ntb row 0)
            rcp = smp.tile([1, 1], f32, tag=f"rcp{b}")
            nc.vector.reciprocal(rcp[:], scani[0:1, KC - 1:KC])
            scal = smp.tile([1, 1], f32, tag=f"scal{b}")
            nc.vector.tensor_tensor(scal[:], ntb_t[0:1, b:b + 1], rcp[:], OP.mult)

            # broadcast offs and scale to 128 partitions (exact fp32 K=1 matmuls)
            psOf = psp.tile([128, KC], f32, tag="acc", name=f"psOf{b}")
            nc.tensor.matmul(psOf[:], ones_row[:], offs[:], start=True, stop=True)
            offs_b = smp.tile([128, KC], f32, tag=f"offsb{b}")
            nc.vector.tensor_copy(offs_b[:], psOf[:])
            psSc = psp.tile([128, 1], f32, tag="acc", name=f"psSc{b}")
            nc.tensor.matmul(psSc[:], ones_row[:], scal[:], start=True, stop=True)
            sc_b = smp.tile([128, 1], f32, tag=f"scb{b}")
            nc.vector.tensor_copy(sc_b[:], psSc[:])

            # full cumsum, scaled and clamped at ntk
            cf = smp.tile([128, KC], f32, tag=f"cf{b}")
            nc.vector.tensor_tensor(cf[:], psC[:], offs_b[:], OP.add)
            ct = smp.tile([128, KC], f32, tag=f"ct{b}")
            nc.vector.tensor_scalar(ct[:], cf[:], sc_b[:], ntb_t[:, b:b + 1],
                                    OP.mult, OP.min)
            ams = smp.tile([128, KC], f32, tag=f"ams{b}")
            nc.vector.tensor_scalar_mul(ams[:], am[:], sc_b[:])
            cm = smp.tile([128, KC], f32, tag=f"cm{b}")
            nc.vector.scalar_tensor_tensor(
                cm[:], cf[:], sc_b[:], ams[:], OP.mult, OP.subtract)
            cpt = smp.tile([128, KC], f32, tag=f"cpt{b}")
            nc.vector.tensor_scalar_min(cpt[:], cm[:], ntb_t[:, b:b + 1])
            ct_l.append(ct)
            cpt_l.append(cpt)

        # ===== einsum / cif / rms interleaved across batches for PE overlap ====
        ht_l = [None] * BPC
        psY_l = [None] * BPC
        y1_l = [None] * BPC
        psS_l = [None] * BPC

        def einsum_phase(b):
            psH = []
            for d in range(DS):
                psH.append(psp.tile([128, NPAD], f32, tag="acc",
                                    name=f"psH{b}_{d}"))
            for c in range(KC):
                audc = audp.tile([128, 1024], f32r, tag="audc",
                                 name=f"audc{b}_{c}")
                for h in range(2):
                    nc.sync.dma_start(
                        audc[64 * h:64 * (h + 1), :],
                        audio[b, 128 * c + 64 * h:128 * c + 64 * (h + 1),
                              :].bitcast(f32r))
                wa = wtp.tile([128, NPAD], f32, tag="wa", name=f"wa{b}_{c}")
                nc.vector.tensor_scalar_min(wa[:], iota_n1[:], ct_l[b][:, c:c + 1])
                wd = wtp.tile([128, NPAD], f32, tag="wd", name=f"wd{b}_{c}")
                nc.vector.scalar_tensor_tensor(
                    wd[:], iota_n[:], cpt_l[b][:, c:c + 1], wa[:],
                    OP.max, OP.subtract)
                wr = wtp.tile([128, NPAD], f32r, tag="wr", bufs=6, name=f"wr{b}_{c}")
                nc.scalar.activation(wr[:], wd[:], AF.Relu, scale=-1.0)
                for d in range(DS):
                    dw = 128 if d < DS - 1 else 127
                    nc.tensor.matmul(
                        psH[d][0:dw, :], audc[:, 128 * d:128 * d + dw], wr[:],
                        start=(c == 0), stop=(c == KC - 1))
            ht = []
            for d in range(DS):
                dw = 128 if d < DS - 1 else 127
                t = htp.tile([128, NPAD], f32r, tag="ht", name=f"ht{b}_{d}")
                nc.vector.tensor_copy(t[0:dw, :], psH[d][0:dw, :])
                ht.append(t)
            ht_l[b] = ht

        def cif_phase(b):
            psY = []
            for e in range(ET):
                p = psp.tile([128, NPAD], f32, tag="acc", name=f"psY{b}_{e}")
                for d in range(DS):
                    dw = 128 if d < DS - 1 else 127
                    nc.tensor.matmul(
                        p[:], cifw_t[d][0:dw, 128 * e:128 * (e + 1)],
                        ht_l[b][d][0:dw, :],
                        start=(d == 0), stop=(d == DS - 1))
                psY.append(p)
            psY_l[b] = psY

        def ss_phase(b):
            y1 = []
            psS = None
            for e in range(ET):
                t = y1p.tile([128, NPAD], f32, tag="y1", name=f"y1_{b}_{e}")
                nc.scalar.activation(t[:], psY_l[b][e][:], AF.Identity,
                                     bias=cifb_cols[:, e:e + 1])
                y1.append(t)
                sq = sqp.tile([128, NPAD], f32r, tag="sq", name=f"sq{b}_{e}")
                nc.scalar.square(sq[:], t[:])
                if e == 0:
                    psS = psp.tile([1, NPAD], f32, tag="acc", name=f"psS{b}")
                nc.tensor.matmul(psS[:], ones_col_r[:], sq[:],
                                 start=(e == 0), stop=(e == ET - 1))
            y1_l[b] = y1
            psS_l[b] = psS
            msq = smp.tile([1, NPAD], f32, tag="msq", name=f"msq{b}")
            nc.vector.tensor_scalar(msq[:], psS[:], 1.0 / 1024.0, RMS_EPS,
                                    OP.mult, OP.add)
            sd = smp.tile([1, NPAD], f32, tag="sd", name=f"sd{b}")
            nc.scalar.activation(sd[:], msq[:], AF.Sqrt)
            rstd_f = smp.tile([1, NPAD], f32, tag="rstdf", name=f"rstdf{b}")
            nc.vector.reciprocal(rstd_f[:], sd[:])
            rstd = smp.tile([1, NPAD], f32r, tag="rstd", name=f"rstd{b}")
            nc.vector.tensor_copy(rstd[:], rstd_f[:])
            return rstd

        def y2_phase(b, rstd):
            psR = psp.tile([128, NPAD], f32, tag="acc", name=f"psR{b}")
            nc.tensor.matmul(psR[:], ones_row_r[:], rstd[:], start=True, stop=True)
            for e in range(ET):
                t = y2p.tile([128, NPAD], f32r, tag="y2", name=f"y2_{b}_{e}")
                nc.vector.scalar_tensor_tensor(
                    t[:], y1_l[b][e][:], lnw_cols[:, e:e + 1], psR[:],
                    OP.mult, OP.mult)
                y2_t[b][e] = t

        einsum_phase(0)
        # cif weights resident (f32r via bitcast byte copy); emitted after the
        # first einsum so its audio-chunk DMAs get the head of the queues
        for d in range(DS):
            dw = 128 if d < DS - 1 else 127
            t = cwp.tile([128, 1024], f32r, tag="cw", name=f"cw{d}")
            hw0 = dw // 2
            nc.sync.dma_start(t[0:hw0, :],
                              cifwT[128 * d:128 * d + hw0, :].bitcast(f32r))
            nc.sync.dma_start(t[hw0:dw, :],
                              cifwT[128 * d + hw0:128 * d + dw, :].bitcast(f32r))
            cifw_t.append(t)
        cif_phase(0)
        rstd0 = ss_phase(0)
        einsum_phase(1)
        y2_phase(0, rstd0)
        cif_phase(1)
        rstd1 = ss_phase(1)

        # ================= phase: text_proj, 512-wide f chunks =================
        NT_TILES = (128, 128, N - 256)
        tw_cache = {}
        bias_cache = {}

        def text_fcg(fcg, bs):
            f0 = 512 * fcg
            if fcg not in tw_cache:
                tw = []
                for e in range(ET):
                    t = twp.tile([128, 512], f32r, tag="tw",
                                 name=f"tw{fcg}_{e}")
                    nc.sync.dma_start(t[:], textwT[128 * e:128 * (e + 1),
                                                   f0:f0 + 512].bitcast(f32r))
                    tw.append(t)
                tw_cache[fcg] = tw
                tbq = twp.tile([1, 512], f32r, tag="tbq", bufs=2,
                               name=f"tbq{fcg}")
                nc.scalar.dma_start(tbq[:], textb[0:1, f0:f0 + 512].bitcast(f32r))
                psB = psp.tile([128, 512], f32, tag="acc", name=f"psB{fcg}")
                nc.tensor.matmul(psB[:], ones_row_r[:], tbq[:],
                                 start=True, stop=True)
                bias_sb = stg.tile([128, 512], f32, tag="bias", bufs=3,
                                   name=f"bias{fcg}")
                nc.vector.tensor_copy(bias_sb[:], psB[:])
                bias_cache[fcg] = bias_sb
            tw = tw_cache[fcg]
            bias_sb = bias_cache[fcg]
            for b in bs:
                for ntile in range(3):
                    nw = NT_TILES[ntile]
                    psO = psp.tile([128, 512], f32, tag="acc",
                                   name=f"psO{fcg}_{b}_{ntile}")
                    for e in range(ET):
                        nc.tensor.matmul(
                            psO[0:nw, :],
                            y2_t[b][e][:, 128 * ntile:128 * ntile + nw],
                            tw[e][:],
                            start=(e == 0), stop=(e == ET - 1))
                    so = stg.tile([128, 512], f32, tag="so",
                                  name=f"so{fcg}_{b}_{ntile}")
                    nc.vector.tensor_tensor(
                        so[0:nw, :], psO[0:nw, :], bias_sb[0:nw, :], OP.add)
                    nc.sync.dma_start(
                        out[b, 128 * ntile:128 * ntile + nw, f0:f0 + 512],
                        so[0:nw, :])

        # first text group for batch 0 runs between ss(1) and y2(1): its
        # matmuls hide the second RMS reciprocal chain on the in-order PE
        text_fcg(0, [0])
        y2_phase(1, rstd1)
        text_fcg(0, [1])
        for fcg in range(1, 8):
            text_fcg(fcg, [0, 1])
    nc.compile()
    return nc


def _get_module():
    global _NC
    if _NC is None:
        _NC = _build_module()
    return _NC


def kernel(**inputs):
    global _last_exec_ns
    from concourse import bass_utils

    audio = np.ascontiguousarray(np.asarray(inputs["audio_features"], dtype=np.float32))
    cif_w = np.asarray(inputs["cif_w"], dtype=np.float32)
    cif_b = np.asarray(inputs["cif_b"], dtype=np.float32)
    ln_w = np.asarray(inputs["ln_w"], dtype=np.float32)
    text_w = np.asarray(inputs["text_w"], dtype=np.float32)
    text_b = np.ascontiguousarray(
        np.asarray(inputs["text_b"], dtype=np.float32).reshape(1, DTXT))
    num_frames = np.asarray(inputs["num_frames"])
    num_text_tokens = np.asarray(inputs["num_text_tokens"])

    cifwT = np.ascontiguousarray(cif_w.T)
    textwT = np.ascontiguousarray(text_w.T)
    cifbpe = np.ascontiguousarray(cif_b.reshape(ET, 128).T)
    lnwpe = np.ascontiguousarray(ln_w.reshape(ET, 128).T)
    nf32 = num_frames.astype(np.float32)
    nt32 = num_text_tokens.astype(np.float32)

    audio_p = np.concatenate(
        [audio, np.zeros((B, TP - T, D), dtype=np.float32)], axis=1)
    # alpha channel, per batch laid out as [128, KC] (partition = t % 128)
    alphaT = np.ascontiguousarray(
        audio_p[:, :, D - 1].reshape(B, KC, 128).transpose(0, 2, 1))

    in_maps = []
    for c in range(NCORES):
        b0 = BPC * c
        nfb = np.ascontiguousarray(
            np.broadcast_to(nf32[b0:b0 + BPC][None, :], (128, BPC)))
        ntb = np.ascontiguousarray(
            np.broadcast_to(nt32[b0:b0 + BPC][None, :], (128, BPC)))
        in_maps.append({
            "audio": np.ascontiguousarray(audio_p[b0:b0 + BPC]),
            "alphaT": np.ascontiguousarray(alphaT[b0:b0 + BPC]),
            "cifwT": cifwT,
            "cifbpe": cifbpe,
            "lnwpe": lnwpe,
            "textwT": textwT,
            "textb": text_b,
            "nfb": nfb,
            "ntb": ntb,
        })

    nc = _get_module()
    trace = bool(int(os.environ.get("KERNEL_TRACE", "0")))
    if trace:
        import sys
        import types
        import trn_agent_boot.trn_boot as tb
        hookmod = types.ModuleType("antenv.axon_hooks")
        hook = tb._ntff_profile_via_ctypes('/opt/axon/libaxon_pjrt.so')
        hookmod.get_axon_ntff_profile_hook = lambda: hook
        sys.modules["antenv.axon_hooks"] = hookmod
    res = bass_utils.run_bass_kernel_spmd(
        nc, in_maps, core_ids=list(range(NCORES)), trace=trace)
    _last_exec_ns = res.exec_time_ns

    out = np.concatenate([r["out"] for r in res.results], axis=0)
    npred = np.concatenate(
        [r["npred"].reshape(BPC) for r in res.results], axis=0)
    return out, num_text_tokens.copy(), npred


# revision 27
# speedup vs baseline: 1.1185x; 1.0367x over previous
"""CFormerAdapter (CIF + cif_proj + RMSNorm + text_proj) on 8 TRN2 NeuronCores.

Sharding: pure data parallel — batch 16 -> 2 per core; weights replicated.

Device algorithm (per core, 2 batch elements):
  The CIF recurrence is replaced by an exact closed form: with C = cumsum(alpha),
  the scatter weights are W[n,t] = relu(min(C_t, n+1) - max(C_{t-1}, n)) with C
  clamped at num_tokens (token nt-1 absorbs the tail; tokens >= nt get 0).
  cumsum is computed as a lower-triangular fp32 matmul per 128-frame chunk plus
  an exclusive scan of chunk totals.  The einsum / cif_proj / text_proj run as
  fp32r (full-rate fp32) matmuls; RMSNorm uses DVE reciprocal + ACT sqrt.
"""

import os
import numpy as np
from contextlib import ExitStack

B, T, D = 16, 1500, 1024
TP = 1536                  # host-padded frame count (12 x 128)
DTXT = 4096
N = 375
NPAD = 384
NCORES = 8
BPC = B // NCORES          # batches per core
KC = 12                    # 128-frame chunks
DS = 8                     # d slices of 128 over 1023 (last is 127)
ET = 8                     # e tiles of 128 over 1024
RMS_EPS = 1e-6

_NC = None
_last_exec_ns = None


def _build_module():
    import concourse.bass as bass
    from concourse import bacc, mybir
    import concourse.tile as tile

    f32 = mybir.dt.float32
    f32r = mybir.dt.float32r
    i32 = mybir.dt.int32
    AF = mybir.ActivationFunctionType
    OP = mybir.AluOpType

    nc = bacc.Bacc("TRN2", target_bir_lowering=False, debug=False)
    audio = nc.dram_tensor("audio", [BPC, TP, D], f32, kind="ExternalInput").ap()
    alphaT = nc.dram_tensor("alphaT", [BPC, 128, KC], f32, kind="ExternalInput").ap()
    cifwT = nc.dram_tensor("cifwT", [1023, 1024], f32, kind="ExternalInput").ap()
    cifbpe = nc.dram_tensor("cifbpe", [128, ET], f32, kind="ExternalInput").ap()
    lnwpe = nc.dram_tensor("lnwpe", [128, ET], f32, kind="ExternalInput").ap()
    textwT = nc.dram_tensor("textwT", [1024, DTXT], f32, kind="ExternalInput").ap()
    textb = nc.dram_tensor("textb", [1, DTXT], f32, kind="ExternalInput").ap()
    nfb = nc.dram_tensor("nfb", [128, BPC], f32, kind="ExternalInput").ap()
    ntb = nc.dram_tensor("ntb", [128, BPC], f32, kind="ExternalInput").ap()
    out = nc.dram_tensor("out", [BPC, N, DTXT], f32, kind="ExternalOutput").ap()
    npred = nc.dram_tensor("npred", [BPC, 1], f32, kind="ExternalOutput").ap()

    with tile.TileContext(nc) as tc, ExitStack() as ctx:
        const = ctx.enter_context(tc.tile_pool(name="const", bufs=1))
        cwp = ctx.enter_context(tc.tile_pool(name="cifw", bufs=8))
        twp = ctx.enter_context(tc.tile_pool(name="textw", bufs=14))
        audp = ctx.enter_context(tc.tile_pool(name="aud", bufs=3))
        wtp = ctx.enter_context(tc.tile_pool(name="wt", bufs=6))
        htp = ctx.enter_context(tc.tile_pool(name="ht", bufs=16))
        y1p = ctx.enter_context(tc.tile_pool(name="y1", bufs=8))
        y2p = ctx.enter_context(tc.tile_pool(name="y2", bufs=16))
        sqp = ctx.enter_context(tc.tile_pool(name="sq", bufs=6))
        smp = ctx.enter_context(tc.tile_pool(name="sm", bufs=1))
        stg = ctx.enter_context(tc.tile_pool(name="stage", bufs=5))
        psp = ctx.enter_context(tc.tile_pool(name="ps", bufs=8, space="PSUM"))

        # ---------------- constants ----------------
        iok = const.tile([128, 128], i32)
        nc.gpsimd.iota(iok[:], pattern=[[0, 128]], base=0, channel_multiplier=1)
        iom = const.tile([128, 128], i32)
        nc.gpsimd.iota(iom[:], pattern=[[1, 128]], base=0, channel_multiplier=0)
        tri = const.tile([128, 128], f32)
        nc.vector.tensor_tensor(tri[:], iok[:], iom[:], OP.is_le)

        iot_i = const.tile([128, KC], i32)
        nc.gpsimd.iota(iot_i[:], pattern=[[128, KC]], base=0, channel_multiplier=1)
        iota_t = const.tile([128, KC], f32)
        nc.vector.tensor_copy(iota_t[:], iot_i[:])

        ion_i = const.tile([128, NPAD], i32)
        nc.gpsimd.iota(ion_i[:], pattern=[[1, NPAD]], base=0, channel_multiplier=0)
        iota_n = const.tile([128, NPAD], f32)
        nc.vector.tensor_copy(iota_n[:], ion_i[:])
        iota_n1 = const.tile([128, NPAD], f32)
        nc.vector.tensor_scalar(iota_n1[:], iota_n[:], 1.0, None, OP.add)

        ones_row = const.tile([1, 128], f32)
        nc.vector.memset(ones_row[:], 1.0)
        ones_row_r = const.tile([1, 128], f32r)
        nc.vector.tensor_copy(ones_row_r[:], ones_row[:])
        ones_col = const.tile([128, 1], f32)
        nc.vector.memset(ones_col[:], 1.0)
        ones_col_r = const.tile([128, 1], f32r)
        nc.vector.tensor_copy(ones_col_r[:], ones_col[:])
        zero_row = const.tile([1, KC], f32)
        nc.vector.memset(zero_row[:], 0.0)

        cifb_cols = const.tile([128, ET], f32)
        nc.sync.dma_start(cifb_cols[:], cifbpe[:])
        lnw_cols = const.tile([128, ET], f32)
        nc.sync.dma_start(lnw_cols[:], lnwpe[:])
        nfb_t = const.tile([128, BPC], f32)
        nc.sync.dma_start(nfb_t[:], nfb[:])
        ntb_t = const.tile([128, BPC], f32)
        nc.sync.dma_start(ntb_t[:], ntb[:])

        # critical-path prep DMAs first (before bulk weight loads)
        at_l = []
        for b in range(BPC):
            at = smp.tile([128, KC], f32, tag=f"at{b}", name=f"at{b}")
            nc.scalar.dma_start(at[:], alphaT[b])
            at_l.append(at)

        y2_t = [[None] * ET for _ in range(BPC)]
        ct_l, cpt_l = [], []
        cifw_t = []

        # ================= phase: alpha prep (both batches) =================
        for b in range(BPC):
            at = at_l[b]
            asig = smp.tile([128, KC], f32, tag=f"asig{b}")
            nc.scalar.activation(asig[:], at[:], AF.Sigmoid)
            am = smp.tile([128, KC], f32, tag=f"am{b}")
            nc.vector.scalar_tensor_tensor(
                am[:], iota_t[:], nfb_t[:, b:b + 1], asig[:], OP.is_lt, OP.mult)

            # in-chunk inclusive prefix via triangular fp32 matmul
            psC = psp.tile([128, KC], f32, tag="acc", name=f"psC{b}")
            nc.tensor.matmul(psC[:], tri[:], am[:], start=True, stop=True)

            # chunk totals (partition 0) -> inclusive scan -> exclusive offsets
            psT = psp.tile([1, KC], f32, tag="acc", name=f"psT{b}")
            nc.tensor.matmul(psT[:], ones_col[:], am[:], start=True, stop=True)
            scani = smp.tile([1, KC], f32, tag=f"scani{b}")
            nc.vector.tensor_tensor_scan(
                scani[:], psT[:], zero_row[:], 0.0, OP.add, OP.add)
            nc.sync.dma_start(npred[b:b + 1, :], scani[0:1, KC - 1:KC])
            offs = smp.tile([1, KC], f32, tag=f"offs{b}")
            nc.vector.memset(offs[0:1, 0:1], 0.0)
            nc.vector.tensor_copy(offs[0:1, 1:KC], scani[0:1, 0:KC - 1])

            # scale = ntk / total  (reciprocal + mult; ntk read from ntb row 0)
            rcp = smp.tile([1, 1], f32, tag=f"rcp{b}")
            nc.vector.reciprocal(rcp[:], scani[0:1, KC - 1:KC])
            scal = smp.tile([1, 1], f32, tag=f"scal{b}")
            nc.vector.tensor_tensor(scal[:], ntb_t[0:1, b:b + 1], rcp[:], OP.mult)

            # broadcast offs and scale to 128 partitions (exact fp32 K=1 matmuls)
            psOf = psp.tile([128, KC], f32, tag="acc", name=f"psOf{b}")
            nc.tensor.matmul(psOf[:], ones_row[:], offs[:], start=True, stop=True)
            offs_b = smp.tile([128, KC], f32, tag=f"offsb{b}")
            nc.vector.tensor_copy(offs_b[:], psOf[:])
            psSc = psp.tile([128, 1], f32, tag="acc", name=f"psSc{b}")
            nc.tensor.matmul(psSc[:], ones_row[:], scal[:], start=True, stop=True)
            sc_b = smp.tile([128, 1], f32, tag=f"scb{b}")
            nc.vector.tensor_copy(sc_b[:], psSc[:])

            # full cumsum, scaled and clamped at ntk
            cf = smp.tile([128, KC], f32, tag=f"cf{b}")
            nc.vector.tensor_tensor(cf[:], psC[:], offs_b[:], OP.add)
            ct = smp.tile([128, KC], f32, tag=f"ct{b}")
            nc.vector.tensor_scalar(ct[:], cf[:], sc_b[:], ntb_t[:, b:b + 1],
                                    OP.mult, OP.min)
            ams = smp.tile([128, KC], f32, tag=f"ams{b}")
            nc.vector.tensor_scalar_mul(ams[:], am[:], sc_b[:])
            cm = smp.tile([128, KC], f32, tag=f"cm{b}")
            nc.vector.scalar_tensor_tensor(
                cm[:], cf[:], sc_b[:], ams[:], OP.mult, OP.subtract)
            cpt = smp.tile([128, KC], f32, tag=f"cpt{b}")
            nc.vector.tensor_scalar_min(cpt[:], cm[:], ntb_t[:, b:b + 1])
            ct_l.append(ct)
            cpt_l.append(cpt)

        # ===== einsum / cif / rms interleaved across batches for PE overlap ====
        ht_l = [None] * BPC
        psY_l = [None] * BPC
        y1_l = [None] * BPC
        psS_l = [None] * BPC

        def einsum_phase(b):
            psH = []
            for d in range(DS):
                psH.append(psp.tile([128, NPAD], f32, tag="acc",
                                    name=f"psH{b}_{d}"))
            for c in range(KC):
                audc = audp.tile([128, 1024], f32r, tag="audc",
                                 name=f"audc{b}_{c}")
                for h in range(2):
                    nc.sync.dma_start(
                        audc[64 * h:64 * (h + 1), :],
                        audio[b, 128 * c + 64 * h:128 * c + 64 * (h + 1),
                              :].bitcast(f32r))
                wa = wtp.tile([128, NPAD], f32, tag="wa", name=f"wa{b}_{c}")
                nc.vector.tensor_scalar_min(wa[:], iota_n1[:], ct_l[b][:, c:c + 1])
                wd = wtp.tile([128, NPAD], f32, tag="wd", name=f"wd{b}_{c}")
                nc.vector.scalar_tensor_tensor(
                    wd[:], iota_n[:], cpt_l[b][:, c:c + 1], wa[:],
                    OP.max, OP.subtract)
                wr = wtp.tile([128, NPAD], f32r, tag="wr", bufs=8, name=f"wr{b}_{c}")
                nc.scalar.activation(wr[:], wd[:], AF.Relu, scale=-1.0)
                for d in range(DS):
                    dw = 128 if d < DS - 1 else 127
                    nc.tensor.matmul(
                        psH[d][0:dw, :], audc[:, 128 * d:128 * d + dw], wr[:],
                        start=(c == 0), stop=(c == KC - 1))
            ht = []
            for d in range(DS):
                dw = 128 if d < DS - 1 else 127
                t = htp.tile([128, NPAD], f32r, tag="ht", name=f"ht{b}_{d}")
                nc.vector.tensor_copy(t[0:dw, :], psH[d][0:dw, :])
                ht.append(t)
            ht_l[b] = ht

        def cif_phase(b):
            psY = []
            for e in range(ET):
                p = psp.tile([128, NPAD], f32, tag="acc", name=f"psY{b}_{e}")
                for d in range(DS):
                    dw = 128 if d < DS - 1 else 127
                    nc.tensor.matmul(
                        p[:], cifw_t[d][0:dw, 128 * e:128 * (e + 1)],
                        ht_l[b][d][0:dw, :],
                        start=(d == 0), stop=(d == DS - 1))
                psY.append(p)
            psY_l[b] = psY

        def ss_phase(b):
            y1 = []
            psS = None
            for e in range(ET):
                t = y1p.tile([128, NPAD], f32, tag="y1", name=f"y1_{b}_{e}")
                nc.scalar.activation(t[:], psY_l[b][e][:], AF.Identity,
                                     bias=cifb_cols[:, e:e + 1])
                y1.append(t)
                sq = sqp.tile([128, NPAD], f32r, tag="sq", name=f"sq{b}_{e}")
                nc.scalar.square(sq[:], t[:])
                if e == 0:
                    psS = psp.tile([1, NPAD], f32, tag="acc", name=f"psS{b}")
                nc.tensor.matmul(psS[:], ones_col_r[:], sq[:],
                                 start=(e == 0), stop=(e == ET - 1))
            y1_l[b] = y1
            psS_l[b] = psS
            msq = smp.tile([1, NPAD], f32, tag="msq", name=f"msq{b}")
            nc.vector.tensor_scalar(msq[:], psS[:], 1.0 / 1024.0, RMS_EPS,
                                    OP.mult, OP.add)
            sd = smp.tile([1, NPAD], f32, tag="sd", name=f"sd{b}")
            nc.scalar.activation(sd[:], msq[:], AF.Sqrt)
            rstd_f = smp.tile([1, NPAD], f32, tag="rstdf", name=f"rstdf{b}")
            nc.vector.reciprocal(rstd_f[:], sd[:])
            rstd = smp.tile([1, NPAD], f32r, tag="rstd", name=f"rstd{b}")
            nc.vector.tensor_copy(rstd[:], rstd_f[:])
            return rstd

        def y2_phase(b, rstd):
            psR = psp.tile([128, NPAD], f32, tag="acc", name=f"psR{b}")
            nc.tensor.matmul(psR[:], ones_row_r[:], rstd[:], start=True, stop=True)
            for e in range(ET):
                t = y2p.tile([128, NPAD], f32r, tag="y2", name=f"y2_{b}_{e}")
                nc.vector.scalar_tensor_tensor(
                    t[:], y1_l[b][e][:], lnw_cols[:, e:e + 1], psR[:],
                    OP.mult, OP.mult)
                y2_t[b][e] = t

        einsum_phase(0)
        # cif weights resident (f32r via bitcast byte copy); emitted after the
        # first einsum so its audio-chunk DMAs get the head of the queues
        for d in range(DS):
            dw = 128 if d < DS - 1 else 127
            t = cwp.tile([128, 1024], f32r, tag="cw", name=f"cw{d}")
            hw0 = dw // 2
            nc.sync.dma_start(t[0:hw0, :],
                              cifwT[128 * d:128 * d + hw0, :].bitcast(f32r))
            nc.sync.dma_start(t[hw0:dw, :],
                              cifwT[128 * d + hw0:128 * d + dw, :].bitcast(f32r))
            cifw_t.append(t)
        cif_phase(0)
        rstd0 = ss_phase(0)
        einsum_phase(1)
        y2_phase(0, rstd0)
        cif_phase(1)
        rstd1 = ss_phase(1)

        # ================= phase: text_proj, 512-wide f chunks =================
        NT_TILES = (128, 128, N - 256)
        tw_cache = {}
        bias_cache = {}

        def text_fcg(fcg, bs):
            f0 = 512 * fcg
            if fcg not in tw_cache:
                tw = []
                for e in range(ET):
                    t = twp.tile([128, 512], f32r, tag="tw",
                                 name=f"tw{fcg}_{e}")
                    nc.sync.dma_start(t[:], textwT[128 * e:128 * (e + 1),
                                                   f0:f0 + 512].bitcast(f32r))
                    tw.append(t)
                tw_cache[fcg] = tw
                tbq = twp.tile([1, 512], f32r, tag="tbq", bufs=2,
                               name=f"tbq{fcg}")
                nc.scalar.dma_start(tbq[:], textb[0:1, f0:f0 + 512].bitcast(f32r))
                psB = psp.tile([128, 512], f32, tag="acc", name=f"psB{fcg}")
                nc.tensor.matmul(psB[:], ones_row_r[:], tbq[:],
                                 start=True, stop=True)
                bias_sb = stg.tile([128, 512], f32, tag="bias", bufs=3,
                                   name=f"bias{fcg}")
                nc.vector.tensor_copy(bias_sb[:], psB[:])
                bias_cache[fcg] = bias_sb
            tw = tw_cache[fcg]
            bias_sb = bias_cache[fcg]
            for b in bs:
                for ntile in range(3):
                    nw = NT_TILES[ntile]
                    psO = psp.tile([128, 512], f32, tag="acc",
                                   name=f"psO{fcg}_{b}_{ntile}")
                    for e in range(ET):
                        nc.tensor.matmul(
                            psO[0:nw, :],
                            y2_t[b][e][:, 128 * ntile:128 * ntile + nw],
                            tw[e][:],
                            start=(e == 0), stop=(e == ET - 1))
                    so = stg.tile([128, 512], f32, tag="so",
                                  name=f"so{fcg}_{b}_{ntile}")
                    nc.vector.tensor_tensor(
                        so[0:nw, :], psO[0:nw, :], bias_sb[0:nw, :], OP.add)
                    nc.sync.dma_start(
                        out[b, 128 * ntile:128 * ntile + nw, f0:f0 + 512],
                        so[0:nw, :])

        # first text group for batch 0 runs between ss(1) and y2(1): its
        # matmuls hide the second RMS reciprocal chain on the in-order PE
        text_fcg(0, [0])
        y2_phase(1, rstd1)
        text_fcg(0, [1])
        for fcg in range(1, 8):
            text_fcg(fcg, [0, 1])
    nc.compile()
    return nc


def _get_module():
    global _NC
    if _NC is None:
        _NC = _build_module()
    return _NC


def kernel(**inputs):
    global _last_exec_ns
    from concourse import bass_utils

    audio = np.ascontiguousarray(np.asarray(inputs["audio_features"], dtype=np.float32))
    cif_w = np.asarray(inputs["cif_w"], dtype=np.float32)
    cif_b = np.asarray(inputs["cif_b"], dtype=np.float32)
    ln_w = np.asarray(inputs["ln_w"], dtype=np.float32)
    text_w = np.asarray(inputs["text_w"], dtype=np.float32)
    text_b = np.ascontiguousarray(
        np.asarray(inputs["text_b"], dtype=np.float32).reshape(1, DTXT))
    num_frames = np.asarray(inputs["num_frames"])
    num_text_tokens = np.asarray(inputs["num_text_tokens"])

    cifwT = np.ascontiguousarray(cif_w.T)
    textwT = np.ascontiguousarray(text_w.T)
    cifbpe = np.ascontiguousarray(cif_b.reshape(ET, 128).T)
    lnwpe = np.ascontiguousarray(ln_w.reshape(ET, 128).T)
    nf32 = num_frames.astype(np.float32)
    nt32 = num_text_tokens.astype(np.float32)

    audio_p = np.concatenate(
        [audio, np.zeros((B, TP - T, D), dtype=np.float32)], axis=1)
    # alpha channel, per batch laid out as [128, KC] (partition = t % 128)
    alphaT = np.ascontiguousarray(
        audio_p[:, :, D - 1].reshape(B, KC, 128).transpose(0, 2, 1))

    in_maps = []
    for c in range(NCORES):
        b0 = BPC * c
        nfb = np.ascontiguousarray(
            np.broadcast_to(nf32[b0:b0 + BPC][None, :], (128, BPC)))
        ntb = np.ascontiguousarray(
            np.broadcast_to(nt32[b0:b0 + BPC][None, :], (128, BPC)))
        in_maps.append({
            "audio": np.ascontiguousarray(audio_p[b0:b0 + BPC]),
            "alphaT": np.ascontiguousarray(alphaT[b0:b0 + BPC]),
            "cifwT": cifwT,
            "cifbpe": cifbpe,
            "lnwpe": lnwpe,
            "textwT": textwT,
            "textb": text_b,
            "nfb": nfb,
            "ntb": ntb,
        })

    nc = _get_module()
    trace = bool(int(os.environ.get("KERNEL_TRACE", "0")))
    if trace:
        import sys
        import types
        import trn_agent_boot.trn_boot as tb
        hookmod = types.ModuleType("antenv.axon_hooks")
        hook = tb._ntff_profile_via_ctypes('/opt/axon/libaxon_pjrt.so')
        hookmod.get_axon_ntff_profile_hook = lambda: hook
        sys.modules["antenv.axon_hooks"] = hookmod
    res = bass_utils.run_bass_kernel_spmd(
        nc, in_maps, core_ids=list(range(NCORES)), trace=trace)
    _last_exec_ns = res.exec_time_ns

    out = np.concatenate([r["out"] for r in res.results], axis=0)
    npred = np.concatenate(
        [r["npred"].reshape(BPC) for r in res.results], axis=0)
    return out, num_text_tokens.copy(), npred
